# revision 1
# baseline (speedup 1.0000x reference)
"""MinLSTM cell (Heinsen-scan reference) as a Bass/Tile kernel for 8 trn2 NeuronCores.

The reference's log-space parallel scan is rewritten exactly in linear space:
    h_t = f'_t h_{t-1} + (1 - f'_t) g(pre_h_t),   h_0 = 1e-6
with f' = sigmoid(pre_f+b_f) / (sigmoid(pre_f+b_f) + sigmoid(pre_i+b_i)) and
g(x) = x>=0 ? x+0.5 : sigmoid(x). This is a convex combination of positive
terms, so it is numerically stable without log-space.

Distribution: data-parallel over batch N=8, one batch element per core, W/b
replicated. Host-side sharding prep transposes x[n] to [H_in, L] fp16 (the
matmul contraction dim must sit on SBUF partitions) and reorders W's output
rows so each 128-channel tile's F/I/H blocks are contiguous. The device
output is [H, L] fp16, transposed/upcast on the host during the gather.

Per-core device pipeline (channel tiles c of 128, sequence chunks of <=2048):
  PE : pre_g = W_g^T @ x^T per gate into 4-bank PSUM tiles (fp16 in, fp32 acc)
  ACT: sf = sigmoid(psF + b_f); si = sigmoid(psI + b_i) (one instr per chunk)
       sh = sigmoid(psH + b_h); rl = relu(psH + b_h)
  DVE: mn   = min(sh, 0.5)                  (tensor_scalar, 4x fp16)
       htil = rl + mn                       (tensor_tensor, 2x fp16; == g exact)
  CCE: s = si += sf                         (SWDGE DMA accumulate, off-engine)
  DVE: r    = reciprocal_approx_fast(s)     (custom op, fp16 out)
       f'   = sf * r                        (tensor_tensor, 2x fp16)
       fpm1 = f' - 1                        (tensor_scalar, 4x fp16)
       wv   = fpm1 * htil                   (tensor_tensor, 2x fp16)
       h    = tensor_tensor_scan(d0=f', d1=wv, op0=mult, op1=subtract)
              => h_t = f'_t h_{t-1} - wv_t, chained across chunks via initial=
  DMA: h chunk -> HBM
The scan is the only sequential op (2 cycles/element on DVE); everything else
is pipelined around it by the Tile scheduler.
"""

import os
import sys

import numpy as np

sys.path.insert(0, "/opt/trn_rl_repo")

import concourse.bass as bass  # noqa: E402
import concourse.tile as tile  # noqa: E402
from concourse import bacc, mybir  # noqa: E402
from concourse.dve_ops import (  # noqa: E402
    RECIP_APPROX_FAST_CONSTS,
    RECIPROCAL_APPROX_FAST,
)

N, L, H_IN, H = 8, 4096, 512, 512
H3 = 3 * H
P = 128
NK = H_IN // P  # 4 k-blocks of the contraction dim
NCT = H // P  # 4 channel tiles
LT = 512  # psum/matmul l-tile (one PSUM bank of fp32)
LH = 2048  # l-granularity of the big DVE ops
NLT = L // LT
NLH = L // LH

F32 = mybir.dt.float32
F16 = mybir.dt.float16
Alu = mybir.AluOpType
Act = mybir.ActivationFunctionType

HX_INIT = 1e-6

_cached_nc = {}


def build_program(L=L, LH=LH):
    key = (L, LH)
    if key in _cached_nc:
        return _cached_nc[key]
    NLH = L // LH

    nc = bacc.Bacc()
    xT_d = nc.dram_tensor("xT", [H_IN, L], F16, kind="ExternalInput")
    wT_d = nc.dram_tensor("wT", [H_IN, H3], F16, kind="ExternalInput")
    bias_d = nc.dram_tensor("bias", [P, 16], F32, kind="ExternalInput")
    out_d = nc.dram_tensor("out", [H, L], F16, kind="ExternalOutput")

    with tile.TileContext(nc) as tc:
        with (
            tc.tile_pool(name="const", bufs=1) as const_pool,
            tc.tile_pool(name="gates", bufs=2) as gates_pool,
            tc.tile_pool(name="sig3", bufs=4) as sig3_pool,
            tc.tile_pool(name="scanbuf", bufs=2) as scan_pool,
            tc.tile_pool(name="psum", bufs=2, space="PSUM") as psum_pool,
        ):
            # Warmup activation with minimal sync deps: absorbs the one-time
            # sigmoid act-table load (walrus rejects table-load + multi-wait
            # on one Activation instruction).
            warm = const_pool.tile([P, 8], F32)
            nc.vector.memset(warm[:], 0.0)
            nc.scalar.activation(warm[:], warm[:], Act.Sigmoid)
            # PE warmup: ~3.5us of garbage matmuls with no dependencies, so the
            # HAM clock gate reaches 2.4GHz while the first DMAs are in flight.
            wup = const_pool.tile([P, P], F16)
            nc.vector.memset(wup[:], 0.0)
            wup_ps = psum_pool.tile([P, P], F32, tag="ps")
            for _ in range(56):
                nc.tensor.matmul(wup_ps[:], wup[:], wup[:], start=True, stop=True)

            xT_sb = const_pool.tile([P, NK, L], F16)
            wT_sb = const_pool.tile([P, NK, H3], F16)
            bias_sb = const_pool.tile([P, 16], F32)

            # W columns are host-reordered grouped by c-tile: each c-tile's
            # F/I/H blocks are contiguous, so the first DMA unblocks c=0.
            # bias rides the sync queue right after the c=0 W group.
            wT_r = wT_d.rearrange("(ki p) o -> p ki o", p=P)
            CW = 3 * P
            for cg in range(NCT):
                nc.sync.dma_start(
                    wT_sb[:, :, cg * CW : (cg + 1) * CW],
                    wT_r[:, :, cg * CW : (cg + 1) * CW],
                )
                if cg == 0:
                    nc.sync.dma_start(bias_sb[:], bias_d[:])
            # x loaded in L-chunks so the first matmuls start early
            xT_r = xT_d.rearrange("(ki p) l -> p ki l", p=P)
            xoff = 0
            if L >= 4096:
                xchunks = [512, 512, 1024] + [2048] * ((L - 2048) // 2048)
            else:
                xchunks = [512] * (L // 512)
            for xch in xchunks:
                nc.scalar.dma_start(
                    xT_sb[:, :, xoff : xoff + xch],
                    xT_r[:, :, xoff : xoff + xch],
                )
                xoff += xch

            # Near-c-major emission with ONE swap: c1's small first chunk is
            # emitted before c0's last chunk, so the in-order PE has c1's
            # first gates ready when DVE drains c0 (hides the c0->c1 seam).
            if L >= 4096:
                clists = {
                    0: [512, 512, 1024] + [2048] * ((L - 2048) // 2048),
                    1: [512, 1536] + [2048] * ((L - 4096) // 2048 + 1),
                    2: [2048] * (L // 2048),
                    3: [2048] * ((L - 2048) // 2048) + [1536, 512],
                }
                order = [(0, 0), (0, 1), (0, 2), (1, 0), (0, 3)]
                order += [(1, j) for j in range(1, len(clists[1]))]
                order += [(2, j) for j in range(len(clists[2]))]
                order += [(3, j) for j in range(len(clists[3]))]
            else:
                clists = {c: [512] * (L // 512) for c in range(NCT)}
                order = [(c, j) for c in range(NCT)
                         for j in range(len(clists[c]))]
            hvs = {}
            lsoff = {c: 0 for c in range(NCT)}
            for c, lh in order:
                if lh == 0:
                    hvs[c] = scan_pool.tile([P, L], F16, tag="hv", name=f"hv{c}")
                hv = hvs[c]
                LHC = clists[c][lh]
                ls = lsoff[c]
                if True:
                    sigf = sig3_pool.tile([P, LHC], F16, tag="sigf")
                    sigi = sig3_pool.tile([P, LHC], F32, tag="sigi")
                    htil = gates_pool.tile([P, LHC], F16, tag="htil")
                    shlh = gates_pool.tile([P, LHC], F16, tag="shlh")
                    rl = gates_pool.tile([P, LHC], F16, tag="rl")

                    # One 4-bank PSUM tile per gate; each gate's sigmoid is a
                    # single full-chunk ACT instruction (less ACT overhead).
                    def gate_mms(ps, ocol):
                        for j in range(LHC // LT):
                            xk = slice(ls + j * LT, ls + (j + 1) * LT)
                            jl = slice(j * LT, (j + 1) * LT)
                            for ki in range(NK):
                                nc.tensor.matmul(
                                    ps[:, jl],
                                    wT_sb[:, ki, ocol : ocol + P],
                                    xT_sb[:, ki, xk],
                                    start=ki == 0,
                                    stop=ki == NK - 1,
                                )

                    def do_F():
                        ps = psum_pool.tile([P, LHC], F32, tag="ps")
                        gate_mms(ps, (c * 3 + 0) * P)
                        nc.scalar.activation(
                            sigf[:], ps[:], Act.Sigmoid,
                            bias=bias_sb[:, 0 * NCT + c : 0 * NCT + c + 1],
                        )

                    def do_I():
                        ps = psum_pool.tile([P, LHC], F32, tag="ps")
                        gate_mms(ps, (c * 3 + 1) * P)
                        nc.scalar.activation(
                            sigi[:], ps[:], Act.Sigmoid,
                            bias=bias_sb[:, 1 * NCT + c : 1 * NCT + c + 1],
                        )

                    def do_H():
                        ps = psum_pool.tile([P, LHC], F32, tag="ps")
                        gate_mms(ps, (c * 3 + 2) * P)
                        nc.scalar.activation(
                            shlh[:], ps[:], Act.Sigmoid,
                            bias=bias_sb[:, 2 * NCT + c : 2 * NCT + c + 1],
                        )
                        return ps

                    do_F()
                    do_I()
                    psH = do_H()
                    # htil = max(x + 0.5, sigmoid(x)), x = psH + b_h
                    # (exact identity for g). t = x + 0.5 via ACT Identity with
                    # per-partition bias; the max is one 2x fp16 TT on DVE.
                    nc.scalar.activation(
                        rl[:], psH[:], Act.Identity,
                        bias=bias_sb[:, 3 * NCT + c : 3 * NCT + c + 1],
                    )

                    nc.vector.tensor_tensor(htil[:], rl[:], shlh[:], Alu.max)
                    # s = sigf + sigi accumulated into sigi via DMA CCE
                    # (first chunk: on DVE, skipping the CCE latency at startup)
                    if (c == 0 and lh <= 2) or (c == 1 and lh == 0):
                        nc.vector.tensor_tensor(sigi[:], sigi[:], sigf[:], Alu.add)
                    else:
                        nc.gpsimd.dma_start(
                            out=sigi[:], in_=sigf[:], accum_op=Alu.add
                        )
                    # reciprocal_approx_fast with fp16 output: the fp32
                    # requirement is on the input bit-trick seed only; emit the
                    # custom op directly so no cast hop is needed.
                    rcp16 = gates_pool.tile([P, LHC], F16, tag="rcp16")
                    _c = RECIP_APPROX_FAST_CONSTS
                    nc.vector._custom_dve(
                        RECIPROCAL_APPROX_FAST, out=rcp16[:], in0=sigi[:],
                        s0=_c["s0"], s1=_c["s1"], imm2=_c["imm2"],
                    )
                    fp = gates_pool.tile([P, LHC], F16, tag="fp")
                    nc.vector.tensor_tensor(fp[:], sigf[:], rcp16[:], Alu.mult)
                    # wv = (fp - 1) * htil as 4x tensor_scalar + 2x fp16 TT
                    fpm1 = gates_pool.tile([P, LHC], F16, tag="fpm1")
                    nc.vector.tensor_scalar_add(fpm1[:], fp[:], -1.0)
                    wv = gates_pool.tile([P, LHC], F16, tag="wv")
                    nc.vector.tensor_tensor(wv[:], fpm1[:], htil[:], Alu.mult)
                    init = HX_INIT if lh == 0 else hv[:, ls - 1 : ls]
                    nc.vector.tensor_tensor_scan(
                        hv[:, ls : ls + LHC], fp[:], wv[:], init,
                        Alu.mult, Alu.subtract,
                    )
                    nc.sync.dma_start(
                        out_d[c * P : (c + 1) * P, ls : ls + LHC],
                        hv[:, ls : ls + LHC],
                    )
                    lsoff[c] += LHC

    nc.compile()
    _cached_nc[key] = nc
    return nc


def _prep_core_inputs(x_n: np.ndarray, wT16: np.ndarray, bias: np.ndarray):
    return {
        "xT": np.ascontiguousarray(x_n.T).astype(np.float16),
        "wT": wT16,
        "bias": bias,
    }


def reorder_w_rows(W: np.ndarray) -> np.ndarray:
    # group output rows by c-tile: [F_c | I_c | H_c] for c = 0..NCT-1
    idx = np.concatenate(
        [np.arange(g * H + c * P, g * H + (c + 1) * P) for c in range(NCT) for g in range(3)]
    )
    return W[idx]


def kernel(x: np.ndarray, W: np.ndarray, b: np.ndarray) -> np.ndarray:
    from concourse.bass_utils import run_bass_kernel_spmd

    nc = build_program()

    wT16 = np.ascontiguousarray(reorder_w_rows(np.asarray(W)).T).astype(np.float16)
    b32 = np.asarray(b, dtype=np.float32)
    bias = np.empty((P, 16), dtype=np.float32)
    for j in range(12):
        bias[:, j] = b32[j * P : (j + 1) * P]
    for c in range(NCT):
        bias[:, 12 + c] = b32[2 * H + c * P : 2 * H + (c + 1) * P] + 0.5

    in_maps = [_prep_core_inputs(np.asarray(x[n]), wT16, bias) for n in range(N)]
    res = run_bass_kernel_spmd(nc, in_maps, list(range(N)))

    out = np.empty((N, L, H), dtype=np.float32)
    for n in range(N):
        out[n] = res.results[n]["out"].T.astype(np.float32)
    return out

